# revision 1
# baseline (speedup 1.0000x reference)
"""GATv2 3-layer GNN (nn_Classifier_GNN) on 8 Trainium2 NeuronCores.

Strategy (edge/graph partitioning):
  - Sort edges by destination node; partition nodes into 8 equal contiguous
    ranges (6250 each) -> each core owns all edges into its node range, so the
    segment softmax + scatter-mean are fully core-local (no all-reduce).
  - Per layer: dense transforms XL = X@Wl / XR = X@Wr are row-sharded (bf16
    matmuls); XL is AllGather'd (src gathers are global), XR stays core-local
    (dst is local).
  - Softmax without max-subtraction (logits are O(5), exp is safe in fp32),
    and the denominator division is pulled out of the segment sum:
       out[n] = (sum_e exp(l_e) * xl_src_e) / (sum_e exp(l_e)) / max(deg,1)
    so one single pass over edges per layer.
  - Per 128-edge tile: xl[src] comes from one 128-row indirect DMA; xr[dst]
    is expanded from the 128-node block on the PE (xr = (A^T)^T @ XR_block);
    the scatter-add onto the node block is another one-hot matmul:
       psum[128n, D] += A^T @ [exp | xl*exp],  A[e,n] = (dstloc[e]==n).
"""

import numpy as np

# ---------------- problem constants (hardcoded; kernel.py is self-contained)
N = 50000
E = 800000
NCORES = 8
P = 128
SH = N // NCORES            # 6250 nodes per core
NBLK = (SH + P - 1) // P    # 49 node blocks per core
PADN = NBLK * P             # 6272
GT = 16                     # tiles per metadata supertile
F1PAD = 256                 # layer-1 input features padded 129 -> 256

# (F_in_padded, heads, out_ch)
LAYERS = [(256, 2, 128), (256, 1, 128), (128, 1, 1)]

_CACHE = {}


# ---------------------------------------------------------------- host prep
def _prep_structure(src, dst):
    """Edge -> (core, supertile, slot, subtile) assignment + per-core arrays."""
    import ml_dtypes
    order = np.argsort(dst, kind="stable")
    ss = src[order].astype(np.int64)
    ds = dst[order].astype(np.int64)
    core = ds // SH
    loc = ds - core * SH
    blk = loc // P
    seg = core * NBLK + blk
    counts = np.bincount(seg, minlength=NCORES * NBLK).reshape(NCORES, NBLK)
    Tb = np.maximum(1, -(-counts // P)).max(axis=0)          # [NBLK]
    tstart = np.concatenate([[0], np.cumsum(Tb)]).astype(np.int64)
    Tsum = int(tstart[-1])
    S = -(-Tsum // GT)
    Ttot = S * GT
    blk_of_tile = np.full(Ttot, NBLK - 1, np.int64)
    for b in range(NBLK):
        blk_of_tile[tstart[b]:tstart[b + 1]] = b
    first_of_blk = np.zeros(Ttot, bool)
    last_of_blk = np.zeros(Ttot, bool)
    for b in range(NBLK):
        lo = int(tstart[b])
        hi = int(tstart[b + 1]) if b < NBLK - 1 else Ttot
        first_of_blk[lo] = True
        last_of_blk[hi - 1] = True

    segstart = np.concatenate([[0], np.cumsum(counts.reshape(-1))])
    rank = np.arange(E, dtype=np.int64) - segstart[seg]
    tile_in_blk = rank // P
    slot = rank % P
    t_glob = tstart[blk] + tile_in_blk
    s_idx = t_glob // GT
    m_idx = t_glob % GT

    sidx = np.zeros((NCORES, S, P, GT), np.int32)
    dstl = np.full((NCORES, S, P, GT), -1.0, ml_dtypes.bfloat16)
    sidx[core, s_idx, slot, m_idx] = ss.astype(np.int32)
    dstl[core, s_idx, slot, m_idx] = (loc - blk * P).astype(ml_dtypes.bfloat16)

    deg = np.bincount(dst, minlength=N).astype(np.float32)
    degr = np.ones((NCORES, P, NBLK), np.float32)
    lrow = np.arange(NBLK)[None, :] * P + np.arange(P)[:, None]     # [P, NBLK]
    valid = lrow < SH                                               # [P, NBLK]
    for r in range(NCORES):
        node = r * SH + lrow
        degr[r][valid] = 1.0 / np.maximum(deg[node[valid]], 1.0)

    return dict(S=S, Ttot=Ttot, blk_of_tile=blk_of_tile,
                first_of_blk=first_of_blk, last_of_blk=last_of_blk,
                sidx=sidx, dstl=dstl, degr=degr)


# ------------------------------------------------------------- bass program
def _build_program(S, blk_of_tile, first_of_blk, last_of_blk, a3,
                   stop_after=None, limit_s=None, repeat=1):
    import concourse.bass as bass
    import concourse.mybir as mybir
    import concourse.tile as tile
    from concourse import bacc
    from concourse.masks import make_identity
    from contextlib import ExitStack

    dt = mybir.dt
    f32 = dt.float32
    bf = dt.bfloat16
    Alu = mybir.AluOpType
    Act = mybir.ActivationFunctionType

    nc = bacc.Bacc("TRN2", target_bir_lowering=False, debug=False,
                   num_devices=NCORES)

    # -------- external inputs (per-core data goes in via in_maps)
    xT = nc.dram_tensor("xT", [F1PAD, PADN], bf, kind="ExternalInput")
    w1 = nc.dram_tensor("w1", [F1PAD, 512], bf, kind="ExternalInput")
    w2 = nc.dram_tensor("w2", [256, 256], bf, kind="ExternalInput")
    w3 = nc.dram_tensor("w3", [128, 2], bf, kind="ExternalInput")
    att1 = nc.dram_tensor("att1", [P, 256], bf, kind="ExternalInput")
    att2 = nc.dram_tensor("att2", [P, 128], bf, kind="ExternalInput")
    iota_in = nc.dram_tensor("iota", [P, P], bf, kind="ExternalInput")
    degr_in = nc.dram_tensor("degr", [P, NBLK], f32, kind="ExternalInput")
    sidx_in = nc.dram_tensor("sidx", [S, P, GT], dt.int32, kind="ExternalInput")
    xsT_in = nc.dram_tensor("xsT", [F1PAD, S * GT * P], bf, kind="ExternalInput")
    dstl_in = nc.dram_tensor("dstl", [S, P, GT], bf, kind="ExternalInput")
    pred_out = nc.dram_tensor("pred", [PADN, 1], f32, kind="ExternalOutput")
    dbg_out = (nc.dram_tensor("dbg", [1024, 512], f32, kind="ExternalOutput")
               if stop_after else None)

    RG = [list(range(NCORES))]

    with ExitStack() as ctx:
        tc = ctx.enter_context(tile.TileContext(nc))
        const = ctx.enter_context(tc.tile_pool(name="const", bufs=1))
        dram = ctx.enter_context(tc.tile_pool(name="dram", bufs=1, space="DRAM"))
        meta_p = ctx.enter_context(tc.tile_pool(name="meta", bufs=4))
        gat_p = ctx.enter_context(tc.tile_pool(name="gather", bufs=12))
        a_p = ctx.enter_context(tc.tile_pool(name="amat", bufs=8))
        sm_p = ctx.enter_context(tc.tile_pool(name="small", bufs=8))
        dn_p = ctx.enter_context(tc.tile_pool(name="dense", bufs=6))
        blk_p = ctx.enter_context(tc.tile_pool(name="blk", bufs=4))
        psum_e = ctx.enter_context(tc.tile_pool(name="psum_e", bufs=2, space="PSUM"))
        psum_d = ctx.enter_context(tc.tile_pool(name="psum_d", bufs=1, space="PSUM"))
        psum_t = ctx.enter_context(tc.tile_pool(name="psum_t", bufs=2, space="PSUM"))
        psum_x = ctx.enter_context(tc.tile_pool(name="psum_x", bufs=3, space="PSUM"))

        # -------- constants to SBUF
        iota_sb = const.tile([P, P], bf)
        nc.sync.dma_start(out=iota_sb, in_=iota_in[:, :])
        att1_sb = const.tile([P, 256], bf)
        nc.sync.dma_start(out=att1_sb, in_=att1[:, :])
        att2_sb = const.tile([P, 128], bf)
        nc.sync.dma_start(out=att2_sb, in_=att2[:, :])
        degr_sb = const.tile([P, NBLK], f32)
        nc.sync.dma_start(out=degr_sb, in_=degr_in[:, :])
        ident = const.tile([P, P], bf)
        make_identity(nc, ident[:, :])
        w1_sb = [const.tile([P, 512], bf, name=f"w1_{k}") for k in range(2)]
        for k in range(2):
            nc.sync.dma_start(out=w1_sb[k], in_=w1[k * P:(k + 1) * P, :])
        w2_sb = [const.tile([P, 256], bf, name=f"w2_{k}") for k in range(2)]
        for k in range(2):
            nc.sync.dma_start(out=w2_sb[k], in_=w2[k * P:(k + 1) * P, :])
        w3_sb = const.tile([P, 2], bf)
        nc.sync.dma_start(out=w3_sb, in_=w3[:, :])

        # -------- DRAM intermediates (fresh per repetition: Shared tensors
        # accept a single writer)
        def alloc_dram(rep):
            xr1 = dram.tile([PADN, 256], bf, name=f"xr1_{rep}")
            xr2 = dram.tile([PADN, 128], bf, name=f"xr2_{rep}")
            xr3 = dram.tile([PADN, 1], bf, name=f"xr3_{rep}")
            xls1 = dram.tile([SH, 256], bf, name=f"xls1_{rep}")
            xls2 = dram.tile([SH, 128], bf, name=f"xls2_{rep}")
            xls3 = dram.tile([SH, 1], bf, name=f"xls3_{rep}")
            xlf1 = dram.tile([N, 256], bf, addr_space="Shared", name=f"xlf1_{rep}")
            xlf2 = dram.tile([N, 128], bf, addr_space="Shared", name=f"xlf2_{rep}")
            xlf3 = dram.tile([N, 1], bf, addr_space="Shared", name=f"xlf3_{rep}")
            h1t = dram.tile([256, PADN], bf, name=f"h1t_{rep}")
            h2t = dram.tile([128, PADN], bf, name=f"h2t_{rep}")
            return xr1, xr2, xr3, xls1, xls2, xls3, xlf1, xlf2, xlf3, h1t, h2t

        xr1, xr2, xr3, xls1, xls2, xls3, xlf1, xlf2, xlf3, h1t, h2t = alloc_dram(0)

        XRS = {1: xr1, 2: xr2, 3: xr3}
        XLS = {1: xls1, 2: xls2, 3: xls3}
        XLF = {1: xlf1, 2: xlf2, 3: xlf3}
        SRC = {1: xT, 2: h1t, 3: h2t}
        WSB = {1: w1_sb, 2: w2_sb, 3: [w3_sb]}
        ATT = {1: att1_sb, 2: att2_sb, 3: None}
        HT = {1: h1t, 2: h2t}

        def dense(l):
            Fp, H, C = LAYERS[l - 1]
            HC = H * C
            OUT = 2 * HC
            nk = Fp // P
            for rt in range(NBLK):
                ps = psum_d.tile([P, OUT], f32, tag="pd", name=f"pd{l}_{rt}")
                for k in range(nk):
                    lt = dn_p.tile([P, P], bf, tag="lt", name=f"lt{l}_{rt}_{k}")
                    nc.sync.dma_start(
                        out=lt, in_=SRC[l][k * P:(k + 1) * P, rt * P:(rt + 1) * P])
                    nc.tensor.matmul(
                        out=ps[:, :], lhsT=lt[:, :], rhs=WSB[l][k][:, :],
                        start=(k == 0), stop=(k == nk - 1))
                osb = dn_p.tile([P, OUT], bf, tag="osb", name=f"osb{l}_{rt}")
                nc.vector.tensor_copy(out=osb[:, :], in_=ps[:, :])
                rows = min(P, SH - rt * P)
                nc.sync.dma_start(out=XLS[l][rt * P:rt * P + rows, :],
                                  in_=osb[:rows, 0:HC])
                nc.sync.dma_start(out=XRS[l][rt * P:(rt + 1) * P, :],
                                  in_=osb[:, HC:OUT])
            nc.gpsimd.collective_compute(
                "AllGather", Alu.bypass, replica_groups=RG,
                ins=[XLS[l][:, :].opt()], outs=[XLF[l][:, :].opt()])

        def postproc(l, b, ps, xrb):
            Fp, H, C = LAYERS[l - 1]
            HC = H * C
            cl = sm_p.tile([P, H], f32, tag="cl", name=f"cl{l}_{b}")
            nc.vector.tensor_scalar(out=cl[:, :], in0=ps[:, 0:H],
                                    scalar1=1e-30, scalar2=None, op0=Alu.max)
            rc = sm_p.tile([P, H], f32, tag="rc", name=f"rc{l}_{b}")
            nc.vector.reciprocal(out=rc[:, :], in_=cl[:, :])
            sc = sm_p.tile([P, H], f32, tag="sc", name=f"sc{l}_{b}")
            nc.vector.tensor_scalar(out=sc[:, :], in0=rc[:, :],
                                    scalar1=degr_sb[:, b:b + 1], scalar2=None,
                                    op0=Alu.mult)
            if l < 3:
                osb = dn_p.tile([P, HC], bf, tag="epost", name=f"ep{l}_{b}")
                if l == 1:
                    # out = msgsum_z*sc - xr*(1/deg): remove the xr part that
                    # rode along in msg = z*exp (sum exp * xr = denom * xr)
                    ndg = sm_p.tile([P, 1], f32, tag="ndg", name=f"ndg{l}_{b}")
                    nc.vector.tensor_scalar(out=ndg[:, :],
                                            in0=degr_sb[:, b:b + 1],
                                            scalar1=-1.0, scalar2=None,
                                            op0=Alu.mult)
                    u = dn_p.tile([P, HC], f32, tag="ucor", name=f"u{l}_{b}")
                    for h in range(H):
                        nc.vector.tensor_scalar(
                            out=u[:, h * C:(h + 1) * C],
                            in0=ps[:, H + h * C:H + (h + 1) * C],
                            scalar1=sc[:, h:h + 1], scalar2=None, op0=Alu.mult)
                    for h in range(H):
                        nc.vector.scalar_tensor_tensor(
                            out=osb[:, h * C:(h + 1) * C],
                            in0=xrb[:, h * C:(h + 1) * C],
                            scalar=ndg[:, 0:1], in1=u[:, h * C:(h + 1) * C],
                            op0=Alu.mult, op1=Alu.add)
                else:
                    for h in range(H):
                        nc.vector.tensor_scalar(
                            out=osb[:, h * C:(h + 1) * C],
                            in0=ps[:, H + h * C:H + (h + 1) * C],
                            scalar1=sc[:, h:h + 1], scalar2=None, op0=Alu.mult)
                for f in range(HC // P):
                    pst = psum_t.tile([P, P], bf, tag="pt", name=f"pt{l}_{b}_{f}")
                    nc.tensor.transpose(out=pst[:, :], in_=osb[:, f * P:(f + 1) * P],
                                        identity=ident[:, :])
                    tsb = dn_p.tile([P, P], bf, tag="tsb", name=f"tsb{l}_{b}_{f}")
                    nc.vector.tensor_copy(out=tsb[:, :], in_=pst[:, :])
                    nc.sync.dma_start(
                        out=HT[l][f * P:(f + 1) * P, b * P:(b + 1) * P],
                        in_=tsb[:, :])
            else:
                prs = sm_p.tile([P, 1], f32, tag="prs", name=f"prs_{b}")
                nc.vector.tensor_scalar(out=prs[:, :], in0=ps[:, 1:2],
                                        scalar1=sc[:, 0:1], scalar2=None,
                                        op0=Alu.mult)
                psg = sm_p.tile([P, 1], f32, tag="psg", name=f"psg_{b}")
                nc.scalar.activation(out=psg[:, :], in_=prs[:, :], func=Act.Sigmoid)
                nc.sync.dma_start(out=pred_out[b * P:(b + 1) * P, :], in_=psg[:, :])

        def edge(l):
            Fp, H, C = LAYERS[l - 1]
            HC = H * C
            D = H + HC
            psum_blocks = {}
            xr_blocks = {}
            for s in range(S if limit_s is None else min(S, limit_s)):
                si = meta_p.tile([P, GT], dt.int32, tag="si", name=f"si{l}_{s}")
                nc.sync.dma_start(out=si[:, :], in_=sidx_in[s])
                dl = meta_p.tile([P, GT], bf, tag="dl", name=f"dl{l}_{s}")
                nc.sync.dma_start(out=dl[:, :], in_=dstl_in[s])

                for m in range(GT):
                    t = s * GT + m
                    b = int(blk_of_tile[t])
                    if first_of_blk[t]:
                        xrb = blk_p.tile([P, HC], bf, tag="xrb", name=f"xrb{l}_{b}")
                        nc.sync.dma_start(out=xrb[:, :],
                                          in_=XRS[l][b * P:(b + 1) * P, :])
                        xr_blocks[b] = xrb
                        psum_blocks[b] = psum_e.tile([P, D], f32, tag="pe",
                                                     name=f"pe{l}_{b}")
                    # one-hot A[e,n] = (dstloc[e] == n), and its transpose
                    A = a_p.tile([P, P], bf, tag="A", name=f"A{l}_{t}")
                    nc.vector.tensor_tensor(
                        out=A[:, :], in0=dl[:, m:m + 1].to_broadcast([P, P]),
                        in1=iota_sb[:, :], op=Alu.is_equal)
                    pat = psum_t.tile([P, P], bf, tag="pt", name=f"pat{l}_{t}")
                    nc.tensor.transpose(out=pat[:, :], in_=A[:, :],
                                        identity=ident[:, :])
                    at = a_p.tile([P, P], bf, tag="at", name=f"at{l}_{t}")
                    nc.vector.tensor_copy(out=at[:, :], in_=pat[:, :])
                    zt = gat_p.tile([P, HC], bf, tag="zt", name=f"zt{l}_{t}")
                    if l == 1:
                        # z = x[src] @ W1l + (A^T)^T @ XR_block, all on the PE
                        pz = psum_x.tile([P, HC], f32, tag="px", name=f"px{l}_{t}")
                        for k in range(2):
                            ltx = dn_p.tile([P, P], bf, tag="ltx",
                                            name=f"ltx{t}_{k}")
                            nc.sync.dma_start(
                                out=ltx,
                                in_=xsT_in[k * P:(k + 1) * P,
                                           t * P:(t + 1) * P])
                            nc.tensor.matmul(out=pz[:, :], lhsT=ltx[:, :],
                                             rhs=w1_sb[k][:, 0:HC],
                                             start=(k == 0), stop=False)
                        nc.tensor.matmul(out=pz[:, :], lhsT=at[:, :],
                                         rhs=xr_blocks[b][:, :],
                                         start=False, stop=True)
                        zraw = gat_p.tile([P, HC], bf, tag="zraw",
                                          name=f"zraw{l}_{t}")
                        nc.vector.tensor_copy(out=zraw[:, :], in_=pz[:, :])
                        nc.vector.scalar_tensor_tensor(
                            out=zt[:, :], in0=zraw[:, :], scalar=0.2,
                            in1=zraw[:, :], op0=Alu.mult, op1=Alu.max)
                    else:
                        # xr tile via PE: xr = (A^T)^T @ XR_block
                        pxr = psum_x.tile([P, HC], f32, tag="px",
                                          name=f"px{l}_{t}")
                        nc.tensor.matmul(out=pxr[:, :], lhsT=at[:, :],
                                         rhs=xr_blocks[b][:, :],
                                         start=True, stop=True)
                        # xl tile: the one indirect gather per tile
                        xlt = gat_p.tile([P, HC], bf, tag="xlt",
                                         name=f"xlt{l}_{t}")
                        nc.gpsimd.indirect_dma_start(
                            out=xlt[:, :], out_offset=None, in_=XLF[l][:, :],
                            in_offset=bass.IndirectOffsetOnAxis(
                                ap=si[:, m:m + 1], axis=0))
                        nc.vector.tensor_tensor(out=zt[:, :], in0=xlt[:, :],
                                                in1=pxr[:, :], op=Alu.add)
                        # leaky relu in one DVE op: z = max(0.2*z, z)
                        nc.vector.scalar_tensor_tensor(
                            out=zt[:, :], in0=zt[:, :], scalar=0.2,
                            in1=zt[:, :], op0=Alu.mult, op1=Alu.max)

                    rhs = gat_p.tile([P, D], bf, tag="rhs", name=f"rhs{l}_{t}")
                    if l < 3:
                        lgt = sm_p.tile([P, H], f32, tag="lgt", name=f"lgt{l}_{t}")
                        scr = a_p.tile([P, HC], bf, tag="scr", name=f"scr{l}_{t}")
                        nc.vector.tensor_tensor(out=scr[:, :], in0=zt[:, :],
                                                in1=ATT[l][:, :], op=Alu.mult)
                        nc.vector.tensor_reduce(
                            out=lgt[:, :],
                            in_=scr[:, :].rearrange("p (h c) -> p h c", c=C),
                            op=Alu.add, axis=mybir.AxisListType.X)
                        ext = sm_p.tile([P, H], f32, tag="ext", name=f"ext{l}_{t}")
                        nc.scalar.activation(out=ext[:, :], in_=lgt[:, :],
                                             func=Act.Exp)
                        nc.vector.tensor_copy(out=rhs[:, 0:H], in_=ext[:, :])
                        # msg = (pre-lrelu operand) * exp.  For layer 1 the
                        # operand is raw z (PSUM) and the xr part is removed
                        # per node in postproc; for l>=2 it is xl directly.
                        msrc = zraw if l == 1 else xlt
                        for h in range(H):
                            nc.scalar.activation(
                                out=rhs[:, H + h * C:H + (h + 1) * C],
                                in_=msrc[:, h * C:(h + 1) * C], func=Act.Copy,
                                scale=ext[:, h:h + 1])
                    else:
                        ext = sm_p.tile([P, 1], f32, tag="ext", name=f"ext{l}_{t}")
                        nc.scalar.activation(out=ext[:, :], in_=zt[:, :],
                                             func=Act.Exp, scale=float(a3))
                        nc.vector.tensor_copy(out=rhs[:, 0:1], in_=ext[:, :])
                        nc.vector.tensor_tensor(out=rhs[:, 1:2], in0=xlt[:, :],
                                                in1=ext[:, :], op=Alu.mult)

                    nc.tensor.matmul(
                        out=psum_blocks[b][:, :], lhsT=A[:, :], rhs=rhs[:, :],
                        start=bool(first_of_blk[t]), stop=bool(last_of_blk[t]))
                    if last_of_blk[t]:
                        postproc(l, b, psum_blocks.pop(b), xr_blocks.pop(b))

        def dump(src_t, rows, cols):
            nc.sync.dma_start(out=dbg_out[:rows, :cols], in_=src_t[:rows, :cols])

        DUMPS = {
            "d1": lambda: dump(xlf1, 1024, 256), "e1": lambda: dump(h1t, 256, 512),
            "d2": lambda: dump(xlf2, 1024, 128), "e2": lambda: dump(h2t, 128, 512),
            "d3": lambda: dump(xlf3, 1024, 1),
            "x1": lambda: dump(xr1, 1024, 256),
        }
        done = False
        for rep in range(repeat):
            if rep > 0:
                (xr1_, xr2_, xr3_, xls1_, xls2_, xls3_,
                 xlf1_, xlf2_, xlf3_, h1t_, h2t_) = alloc_dram(rep)
                XRS.update({1: xr1_, 2: xr2_, 3: xr3_})
                XLS.update({1: xls1_, 2: xls2_, 3: xls3_})
                XLF.update({1: xlf1_, 2: xlf2_, 3: xlf3_})
                SRC.update({2: h1t_, 3: h2t_})
                HT.update({1: h1t_, 2: h2t_})
            for l in (1, 2, 3):
                if done:
                    break
                dense(l)
                if stop_after == f"d{l}" or stop_after == "x1":
                    DUMPS[stop_after]()
                    done = True
                    break
                edge(l)
                if stop_after == f"e{l}":
                    DUMPS[stop_after]()
                    done = True
                    break
            if done:
                break

    nc.compile()
    return nc


# ------------------------------------------------------------------ driver
def _host_inputs(inputs, st):
    import ml_dtypes
    bf = ml_dtypes.bfloat16
    x = np.asarray(inputs["x"], np.float32)
    a1 = np.asarray(inputs["a1"], np.float32)
    a2 = np.asarray(inputs["a2"], np.float32)

    xT = np.zeros((NCORES, F1PAD, PADN), bf)
    for r in range(NCORES):
        xT[r, :129, :SH] = x[r * SH:(r + 1) * SH].T.astype(bf)
    w1 = np.zeros((F1PAD, 512), bf)
    w1[:129, 0:256] = np.asarray(inputs["W1l"], np.float32).astype(bf)
    w1[:129, 256:512] = np.asarray(inputs["W1r"], np.float32).astype(bf)
    w2 = np.concatenate([np.asarray(inputs["W2l"], np.float32),
                         np.asarray(inputs["W2r"], np.float32)],
                        axis=1).astype(bf)
    w3 = np.concatenate([np.asarray(inputs["W3l"], np.float32),
                         np.asarray(inputs["W3r"], np.float32)],
                        axis=1).astype(bf)
    att1 = np.tile(a1.reshape(1, -1), (P, 1)).astype(bf)
    att2 = np.tile(a2.reshape(1, -1), (P, 1)).astype(bf)
    iota = np.tile(np.arange(P, dtype=np.float32)[None, :], (P, 1)).astype(bf)

    # expanded x[src] per edge-tile column, transposed: [256, Ttot*128]
    xb = np.zeros((F1PAD, N), bf)
    xb[:129, :] = x.T.astype(bf)
    in_maps = []
    for r in range(NCORES):
        col_src = st["sidx"][r].transpose(0, 2, 1).reshape(-1)   # [Ttot*P]
        xsT = np.ascontiguousarray(xb[:, col_src])
        in_maps.append({
            "xT": np.ascontiguousarray(xT[r]), "xsT": xsT,
            "w1": w1, "w2": np.ascontiguousarray(w2),
            "w3": np.ascontiguousarray(w3),
            "att1": att1, "att2": att2, "iota": iota,
            "degr": np.ascontiguousarray(st["degr"][r]),
            "sidx": np.ascontiguousarray(st["sidx"][r]),
            "dstl": np.ascontiguousarray(st["dstl"][r]),
        })
    return in_maps


def kernel(x, edge_index, y, train_idx,
           W1l, W1r, a1, b1, W2l, W2r, a2, b2, W3l, W3r, a3, b3,
           _return_timing=None):
    from concourse.bass_utils import run_bass_kernel_spmd

    inputs = dict(x=x, edge_index=edge_index, y=y, train_idx=train_idx,
                  W1l=W1l, W1r=W1r, a1=a1, W2l=W2l, W2r=W2r, a2=a2,
                  W3l=W3l, W3r=W3r, a3=a3)
    ei = np.asarray(edge_index)
    src, dst = ei[0].astype(np.int64), ei[1].astype(np.int64)

    key = (hash(src.tobytes()) ^ hash(dst.tobytes()),
           float(np.asarray(a3).ravel()[0]))
    if key not in _CACHE:
        st = _prep_structure(src, dst)
        nc = _build_program(st["S"], st["blk_of_tile"], st["first_of_blk"],
                            st["last_of_blk"], float(np.asarray(a3).ravel()[0]))
        _CACHE[key] = (st, nc)
    st, nc = _CACHE[key]

    in_maps = _host_inputs(inputs, st)
    res = run_bass_kernel_spmd(nc, in_maps, core_ids=list(range(NCORES)),
                               **(_return_timing or {}))
    pred = np.concatenate([res.results[r]["pred"][:SH, 0] for r in range(NCORES)])
    ti = np.asarray(train_idx)
    if _return_timing is not None:
        kernel._last_result = res
    return (pred[ti].astype(np.float32),
            np.asarray(y, np.float32)[ti].astype(np.float32))

